# revision 8
# baseline (speedup 1.0000x reference)
"""ContentOnlyPhasorBlock on 8 Trainium2 NeuronCores.

Math: the reference is causal linear attention in disguise.
  phi_k = [amp*cos(kp), amp*sin(kp)]  (L, 2K=128)
  phi_q = [amp*cos(qp), amp*sin(qp)]
  retrieved[l] = sum_{t<=l} (phi_q[l] . phi_k[t]) V[t]
The per-row 1/sqrt((l+1)K) norm is absorbed by the LayerNorm (scale
invariance); only the eps term needs rescaling: eps' = eps*(l+1)*K.
ln_g/ln_b/out_b fold into out_w on the host.

Sharding: sequence-parallel, 256 rows per core. Each core computes its
own MLPs + chunk state S_i = phi_k_i^T @ V_i (128x512), one AllGather
of the 8 states, prefix-sum via per-core 0/1-diagonal matmuls, then
intra-chunk quadratic attention + inter-chunk via the prefix state.

All matmuls run in float32r (fp32 with 11-bit mantissa, 4x faster than
fp32 on the PE): inputs from DRAM are pre-rounded on the host; on-chip
producers write f32r tiles (HW rounds on write). Residual x is added in
full fp32 on the DVE.
"""
import sys
if '/opt/trn_rl_repo' not in sys.path:
    sys.path.insert(0, '/opt/trn_rl_repo')
import math
import numpy as np
import concourse.bass as bass
import concourse.bacc as bacc
import concourse.mybir as mybir
import concourse.tile as tile
from concourse.bass_utils import run_bass_kernel_spmd

AF = mybir.ActivationFunctionType
ALU = mybir.AluOpType
F32 = mybir.dt.float32
F32R = mybir.dt.float32r

B, L, D, K = 1, 2048, 512, 64
NCORES = 8
R = L // NCORES          # 256 rows per core
NB = R // 128            # 2 l-blocks
ND = D // 128            # 4 d-tiles

RUN_KWARGS = {}          # test harness can inject trace=True etc.
LAST_RESULTS = None
_PROGRAM_CACHE = {}


def _fp32r_round(x):
    u = np.ascontiguousarray(x, np.float32).view(np.uint32).astype(np.uint64)
    u = (u + 0x800) & 0xFFFFF000
    return (u & 0xFFFFFFFF).astype(np.uint32).view(np.float32)


def _build_program():
    nc = bacc.Bacc("TRN2", target_bir_lowering=False, debug=False,
                   num_devices=NCORES)

    # ---------------- DRAM I/O ----------------
    din = {}
    def inp(name, shape, dt=F32R):
        din[name] = nc.dram_tensor(name, list(shape), dt, kind="ExternalInput")
        return din[name]

    xT_d = inp("xT", [D, R])                 # x chunk transposed (rounded)
    x_d = inp("x_rm", [R, D], F32)           # residual, full fp32
    kw1_d = inp("ke_w1", [D, D])
    qw1_d = inp("qe_w1", [D, D])
    vw_d = inp("v_w", [D, D])
    ow_d = inp("w_eff", [D, D])
    w2k_d = inp("w2k", [D, 128])             # ke_w2 duplicated on cols
    w2q_d = inp("w2q", [D, 128])
    wam_d = inp("wamp", [D, 128])            # amp_w duplicated
    b1k_d = inp("b1k", [1, D])
    b1q_d = inp("b1q", [1, D])
    b2k_d = inp("b2k", [1, 128])
    b2q_d = inp("b2q", [1, 128])
    bam_d = inp("bamp", [1, 128])
    vb_d = inp("vb", [1, D])
    ob_d = inp("ob", [1, D])
    id_d = inp("ident", [128, 128])
    mask_d = inp("mask", [128, 128], F32)    # m[t,l] = 1 if l >= t
    wdg_d = inp("wdiag", [128, 7 * 128])     # per-core prefix 0/1 diagonals
    eps_d = inp("epsvec", [NB, 128], F32)    # 1e-5 * K * (gl+1), per l-block
    ones_d = inp("ones_r", [1, D])

    y_d = nc.dram_tensor("y", [R, D], F32, kind="ExternalOutput")

    with tile.TileContext(nc) as tc:
        with tc.tile_pool(name="sb", bufs=1) as sb, \
             tc.tile_pool(name="ps", bufs=1, space="PSUM") as ps, \
             tc.tile_pool(name="dr", bufs=1, space="DRAM") as dr:

            # ---------------- constants / small vectors ----------------
            ones_r = sb.tile([1, D], F32R, name="ones_r")
            nc.sync.dma_start(ones_r[:], ones_d[:])
            sinsc = sb.tile([128, 1], F32, name="sinsc")
            nc.gpsimd.memset(sinsc[0:64, :], -math.pi)
            nc.gpsimd.memset(sinsc[64:128, :], math.pi)
            sinbs = sb.tile([128, 1], F32, name="sinbs")
            nc.gpsimd.memset(sinbs[0:64, :], math.pi / 2)
            nc.gpsimd.memset(sinbs[64:128, :], 0.0)

            # ---------------- input loads ----------------
            def load_tiles(name, dram, p, f, n, dt=F32R):
                ts = []
                for t in range(n):
                    tl = sb.tile([p, f], dt, name=f"{name}{t}")
                    nc.sync.dma_start(tl[:], dram[t * p:(t + 1) * p, :])
                    ts.append(tl)
                return ts

            xT = load_tiles("xT", xT_d, 128, R, ND)
            kw1 = load_tiles("kw1", kw1_d, 128, D, ND)
            w2k = load_tiles("w2k", w2k_d, 128, 128, ND)
            wam = load_tiles("wam", wam_d, 128, 128, ND)
            vw = load_tiles("vw", vw_d, 128, D, ND)
            id_sb = sb.tile([128, 128], F32R, name="id_sb")
            nc.sync.dma_start(id_sb[:], id_d[:])
            b1k = sb.tile([1, D], F32R, name="b1k_sb")
            nc.sync.dma_start(b1k[:], b1k_d[:])
            b2k = sb.tile([1, 128], F32R, name="b2k_sb")
            nc.sync.dma_start(b2k[:], b2k_d[:])
            bam = sb.tile([1, 128], F32R, name="bam_sb")
            nc.sync.dma_start(bam[:], bam_d[:])
            vb = sb.tile([1, D], F32R, name="vb_sb")
            nc.sync.dma_start(vb[:], vb_d[:])
            qw1 = load_tiles("qw1", qw1_d, 128, D, ND)
            w2q = load_tiles("w2q", w2q_d, 128, 128, ND)
            b1q = sb.tile([1, D], F32R, name="b1q_sb")
            nc.sync.dma_start(b1q[:], b1q_d[:])
            b2q = sb.tile([1, 128], F32R, name="b2q_sb")
            nc.sync.dma_start(b2q[:], b2q_d[:])
            mask = sb.tile([128, 128], F32, name="mask_sb")
            nc.sync.dma_start(mask[:], mask_d[:])
            wdg = sb.tile([128, 7 * 128], F32R, name="wdg_sb")
            nc.sync.dma_start(wdg[:], wdg_d[:])
            ow = load_tiles("ow", ow_d, 128, D, ND)
            ob = sb.tile([1, D], F32R, name="ob_sb")
            nc.sync.dma_start(ob[:], ob_d[:])
            x_rm = load_tiles("x_rm", x_d, 128, D, NB, dt=F32)
            epsv = []
            for lb in range(NB):
                ev = sb.tile([128, 1], F32, name=f"epsv{lb}")
                nc.sync.dma_start(ev[:], eps_d[lb:lb+1, :].rearrange("a b -> b a"))
                epsv.append(ev)

            # ---------------- k/v path ----------------
            # hkq[dout]: [128, 512] = [gelu(ke): l 0:256 | gelu(qe): l 256:512]
            hkq = []
            for do in range(ND):
                h_ps = ps.tile([128, D], F32, name=f"h_ps{do}", tag="acc", bufs=2)
                for dj in range(ND):
                    nc.tensor.matmul(h_ps[:, 0:R], kw1[dj][:, do*128:(do+1)*128],
                                     xT[dj][:], start=(dj == 0), stop=False)
                nc.tensor.matmul(h_ps[:, 0:R], b1k[:, do*128:(do+1)*128],
                                 ones_r[:, 0:R], start=False, stop=True,
                                 skip_group_check=True)
                for dj in range(ND):
                    nc.tensor.matmul(h_ps[:, R:2*R], qw1[dj][:, do*128:(do+1)*128],
                                     xT[dj][:], start=(dj == 0), stop=False)
                nc.tensor.matmul(h_ps[:, R:2*R], b1q[:, do*128:(do+1)*128],
                                 ones_r[:, 0:R], start=False, stop=True,
                                 skip_group_check=True)
                h_sb = sb.tile([128, D], F32R, name=f"hkq{do}")
                nc.scalar.activation(h_sb[:], h_ps[:], AF.Gelu)
                hkq.append(h_sb)

            # ---------------- phases ----------------
            ph_ps = ps.tile([128, D], F32, name="ph_ps", tag="acc", bufs=2)
            for dj in range(ND):
                nc.tensor.matmul(ph_ps[:, 0:R], w2k[dj][:], hkq[dj][:, 0:R],
                                 start=(dj == 0), stop=False)
            nc.tensor.matmul(ph_ps[:, 0:R], b2k[:], ones_r[:, 0:R],
                             start=False, stop=True, skip_group_check=True)
            for dj in range(ND):
                nc.tensor.matmul(ph_ps[:, R:2*R], w2q[dj][:], hkq[dj][:, R:2*R],
                                 start=(dj == 0), stop=False)
            nc.tensor.matmul(ph_ps[:, R:2*R], b2q[:], ones_r[:, 0:R],
                             start=False, stop=True, skip_group_check=True)
            t_sb = sb.tile([128, D], F32, name="t_sb")
            nc.scalar.activation(t_sb[:], ph_ps[:], AF.Tanh)
            nc.scalar.activation(t_sb[0:64, :], t_sb[0:64, :], AF.Abs)
            cs = sb.tile([128, D], F32, name="cs_sb")
            nc.scalar.activation(cs[:], t_sb[:], AF.Sin, bias=sinbs[:], scale=sinsc[:])

            # ---------------- amp ----------------
            am_ps = ps.tile([128, R], F32, name="am_ps", tag="sm", bufs=2)
            for dj in range(ND):
                nc.tensor.matmul(am_ps[:], wam[dj][:], xT[dj][:],
                                 start=(dj == 0), stop=False)
            nc.tensor.matmul(am_ps[:], bam[:], ones_r[:, 0:R],
                             start=False, stop=True, skip_group_check=True)
            e_sb = sb.tile([128, R], F32, name="e_sb")
            nc.scalar.activation(e_sb[:], am_ps[:], AF.Exp)
            e1_sb = sb.tile([128, R], F32, name="e1_sb")
            nc.vector.tensor_scalar_add(e1_sb[:], e_sb[:], 1.0)
            al_sb = sb.tile([128, R], F32, name="al_sb")
            nc.scalar.activation(al_sb[:], e1_sb[:], AF.Ln)

            # phi tiles: [kr;ki] and [qr;qi], feature-major [128, 256]
            phik = sb.tile([128, R], F32R, name="phik")
            nc.vector.scalar_tensor_tensor(phik[:], al_sb[:], 0.1, cs[:, 0:R],
                                           ALU.add, ALU.mult)
            phiq = sb.tile([128, R], F32R, name="phiq")
            nc.vector.scalar_tensor_tensor(phiq[:], al_sb[:], 0.1, cs[:, R:2*R],
                                           ALU.add, ALU.mult)

            # ---------------- V ----------------
            V_sb = []
            for lb in range(NB):
                v_ps = ps.tile([128, D], F32, name=f"v_ps{lb}", tag="acc", bufs=2)
                for dj in range(ND):
                    nc.tensor.matmul(v_ps[:], xT[dj][:, lb*128:(lb+1)*128],
                                     vw[dj][:], start=(dj == 0), stop=False)
                nc.tensor.matmul(v_ps[:], ones_r[:, 0:128], vb[:], start=False,
                                 stop=True, skip_group_check=True)
                v_sb = sb.tile([128, D], F32R, name=f"V{lb}")
                nc.scalar.copy(v_sb[:], v_ps[:])
                V_sb.append(v_sb)

            # ---------------- chunk state S + AllGather ----------------
            phik_rm = []
            for tb in range(NB):
                tr_ps = ps.tile([128, 128], F32, name=f"ktr_ps{tb}", tag="tr", bufs=2)
                nc.tensor.matmul(tr_ps[:], phik[:, tb*128:(tb+1)*128], id_sb[:],
                                 start=True, stop=True)
                k_rm = sb.tile([128, 128], F32R, name=f"phik_rm{tb}")
                nc.vector.tensor_copy(k_rm[:], tr_ps[:])
                phik_rm.append(k_rm)
            s_ps = ps.tile([128, D], F32, name="s_ps", tag="acc", bufs=2)
            for tb in range(NB):
                nc.tensor.matmul(s_ps[:], phik_rm[tb][:], V_sb[tb][:],
                                 start=(tb == 0), stop=(tb == NB - 1))
            s_sb = sb.tile([128, D], F32R, name="s_sb")
            nc.scalar.copy(s_sb[:], s_ps[:])
            cc_in = dr.tile([128, D], F32R, name="cc_in")
            cc_out = dr.tile([NCORES, 128, D], F32R, addr_space="Shared",
                             name="cc_out")
            nc.sync.dma_start(cc_in[:], s_sb[:])
            nc.gpsimd.collective_compute(
                "AllGather", ALU.bypass,
                replica_groups=[list(range(NCORES))],
                ins=[cc_in[:]], outs=[cc_out[:]],
            )

            # ---------------- intra-chunk scores (overlap AG) ----------------
            a_m = {}
            for tb in range(NB):
                a_ps = ps.tile([128, R], F32, name=f"a_ps{tb}", tag="sm", bufs=2)
                nc.tensor.matmul(a_ps[:], phik[:, tb*128:(tb+1)*128], phiq[:],
                                 start=True, stop=True)
                if tb == 0:
                    a00 = sb.tile([128, 128], F32R, name="a00")
                    nc.vector.tensor_tensor(a00[:], a_ps[:, 0:128], mask[:], ALU.mult)
                    a01 = sb.tile([128, 128], F32R, name="a01")
                    nc.vector.tensor_copy(a01[:], a_ps[:, 128:256])
                    a_m[(0, 0)], a_m[(0, 1)] = a00, a01
                else:
                    a11 = sb.tile([128, 128], F32R, name="a11")
                    nc.vector.tensor_tensor(a11[:], a_ps[:, 128:256], mask[:], ALU.mult)
                    a_m[(1, 1)] = a11

            # ---------------- prefix state P ----------------
            s_all = []
            for j in range(NCORES - 1):
                sa = sb.tile([128, D], F32R, name=f"s_all{j}")
                nc.sync.dma_start(sa[:], cc_out[j])
                s_all.append(sa)
            p_ps = ps.tile([128, D], F32, name="p_ps", tag="acc", bufs=2)
            for j in range(NCORES - 1):
                nc.tensor.matmul(p_ps[:], wdg[:, j*128:(j+1)*128], s_all[j][:],
                                 start=(j == 0), stop=(j == NCORES - 2))
            p_sb = sb.tile([128, D], F32R, name="p_sb")
            nc.scalar.copy(p_sb[:], p_ps[:])

            # ---------------- retrieve + LN + out per l-block ----------------
            for lb in range(NB):
                r_ps = ps.tile([128, D], F32, name=f"r_ps{lb}", tag="racc", bufs=2)
                first = True
                for tb in range(lb + 1):
                    nc.tensor.matmul(r_ps[:], a_m[(tb, lb)][:], V_sb[tb][:],
                                     start=first, stop=False)
                    first = False
                nc.tensor.matmul(r_ps[:], phiq[:, lb*128:(lb+1)*128], p_sb[:],
                                 start=False, stop=True, skip_group_check=True)
                # LayerNorm stats (eps absorbs the 1/sqrt((l+1)K) row norm)
                bn6 = sb.tile([128, 6], F32, name=f"bn6_{lb}")
                nc.vector.bn_stats(bn6[:], r_ps[:])
                bn2 = sb.tile([128, 2], F32, name=f"bn2_{lb}")
                nc.vector.bn_aggr(bn2[:], bn6[:])
                lnv = sb.tile([128, 1], F32, name=f"lnv{lb}")
                nc.scalar.activation(lnv[:], bn2[:, 1:2], AF.Ln,
                                     bias=epsv[lb][:], scale=1.0)
                rstd = sb.tile([128, 1], F32, name=f"rstd{lb}")
                nc.scalar.activation(rstd[:], lnv[:], AF.Exp, bias=0.0, scale=-0.5)
                nmu = sb.tile([128, 1], F32, name=f"nmu{lb}")
                nc.vector.tensor_scalar_mul(nmu[:], bn2[:, 0:1], -1.0)
                s2v = sb.tile([128, 1], F32, name=f"s2v{lb}")
                nc.vector.tensor_tensor(s2v[:], nmu[:], rstd[:], ALU.mult)
                z_sb = sb.tile([128, D], F32R, name=f"z{lb}")
                nc.vector.tensor_scalar(z_sb[:], r_ps[:], rstd[:], s2v[:],
                                        ALU.mult, ALU.add)
                # transpose z, out-proj, bias, residual
                o_ps = ps.tile([128, D], F32, name=f"o_ps{lb}", tag="racc", bufs=2)
                for dt in range(ND):
                    zt_ps = ps.tile([128, 128], F32, name=f"zt_ps{lb}_{dt}",
                                    tag="tr", bufs=2)
                    nc.tensor.matmul(zt_ps[:], z_sb[:, dt*128:(dt+1)*128],
                                     id_sb[:], start=True, stop=True)
                    zt_sb = sb.tile([128, 128], F32R, name=f"zt{lb}_{dt}")
                    if dt % 2 == 0:
                        nc.vector.tensor_copy(zt_sb[:], zt_ps[:])
                    else:
                        nc.scalar.copy(zt_sb[:], zt_ps[:])
                    nc.tensor.matmul(o_ps[:], zt_sb[:], ow[dt][:],
                                     start=(dt == 0), stop=False,
                                     skip_group_check=True)
                nc.tensor.matmul(o_ps[:], ones_r[:, 0:128], ob[:], start=False,
                                 stop=True, skip_group_check=True)
                y_sb = sb.tile([128, D], F32, name=f"y{lb}")
                nc.vector.tensor_tensor(y_sb[:], o_ps[:], x_rm[lb][:], ALU.add)
                nc.sync.dma_start(y_d[lb*128:(lb+1)*128, :], y_sb[:])

    nc.compile()
    return nc


def kernel(**inputs):
    global LAST_RESULTS
    if 'prog' not in _PROGRAM_CACHE:
        _PROGRAM_CACHE['prog'] = _build_program()
    nc = _PROGRAM_CACHE['prog']

    f = {k: np.asarray(v, np.float32) for k, v in inputs.items()}
    x = f['x'][0]                                   # (L, D)
    rr = _fp32r_round
    W_eff = rr(f['ln_g'][:, None] * f['out_w'])
    b_eff = rr((f['ln_b'] @ f['out_w'] + f['out_b'])[None, :])
    shared = {
        "ke_w1": rr(f['ke_w1']), "qe_w1": rr(f['qe_w1']),
        "v_w": rr(f['v_w']), "w_eff": W_eff,
        "w2k": rr(np.concatenate([f['ke_w2'], f['ke_w2']], 1)),
        "w2q": rr(np.concatenate([f['qe_w2'], f['qe_w2']], 1)),
        "wamp": rr(np.concatenate([f['amp_w'], f['amp_w']], 1)),
        "b1k": rr(f['ke_b1'][None, :]), "b1q": rr(f['qe_b1'][None, :]),
        "b2k": rr(np.concatenate([f['ke_b2'], f['ke_b2']])[None, :]),
        "b2q": rr(np.concatenate([f['qe_b2'], f['qe_b2']])[None, :]),
        "bamp": rr(np.concatenate([f['amp_b'], f['amp_b']])[None, :]),
        "vb": rr(f['v_b'][None, :]), "ob": b_eff,
        "ident": np.eye(128, dtype=np.float32),
        "ones_r": np.ones((1, D), np.float32),
        "mask": (np.arange(128)[None, :] >= np.arange(128)[:, None]
                 ).astype(np.float32),
    }
    in_maps = []
    for c in range(NCORES):
        xc = x[R*c:R*(c+1)]
        wdiag = np.zeros((128, 7 * 128), np.float32)
        for j in range(min(c, 7)):
            wdiag[:, j*128:(j+1)*128] = np.eye(128, dtype=np.float32)
        gl = np.arange(R*c, R*(c+1), dtype=np.float64)
        in_maps.append({
            **shared,
            "xT": rr(np.ascontiguousarray(xc.T)),
            "x_rm": np.ascontiguousarray(xc),
            "wdiag": wdiag,
            "epsvec": (1e-5 * K * (gl + 1)).astype(np.float32).reshape(NB, 128),
        })

    res = run_bass_kernel_spmd(nc, in_maps, core_ids=list(range(NCORES)),
                               **RUN_KWARGS)
    LAST_RESULTS = res
    y = np.concatenate([res.results[c]['y'] for c in range(NCORES)], axis=0)
    return y[None].astype(np.float32)


# revision 10
# speedup vs baseline: 1.0093x; 1.0093x over previous
"""ContentOnlyPhasorBlock on 8 Trainium2 NeuronCores.

Math: the reference is causal linear attention in disguise.
  phi_k = [amp*cos(kp), amp*sin(kp)]  (L, 2K=128)
  phi_q = [amp*cos(qp), amp*sin(qp)]
  retrieved[l] = sum_{t<=l} (phi_q[l] . phi_k[t]) V[t]
The per-row 1/sqrt((l+1)K) norm is absorbed by the LayerNorm (scale
invariance); only the eps term needs rescaling: eps' = eps*(l+1)*K.
ln_g/ln_b/out_b fold into out_w on the host.

Sharding: sequence-parallel, 256 rows per core. Each core computes its
own MLPs + chunk state S_i = phi_k_i^T @ V_i (128x512), one AllGather
of the 8 states, prefix-sum via per-core 0/1-diagonal matmuls, then
intra-chunk quadratic attention + inter-chunk via the prefix state.

All matmuls run in float32r (fp32 with 11-bit mantissa, 4x faster than
fp32 on the PE): inputs from DRAM are pre-rounded on the host; on-chip
producers write f32r tiles (HW rounds on write). Residual x is added in
full fp32 on the DVE.
"""
import sys
if '/opt/trn_rl_repo' not in sys.path:
    sys.path.insert(0, '/opt/trn_rl_repo')
import math
import numpy as np
import concourse.bass as bass
import concourse.bacc as bacc
import concourse.mybir as mybir
import concourse.tile as tile
from concourse.bass_utils import run_bass_kernel_spmd

AF = mybir.ActivationFunctionType
ALU = mybir.AluOpType
F32 = mybir.dt.float32
F32R = mybir.dt.float32r

B, L, D, K = 1, 2048, 512, 64
NCORES = 8
R = L // NCORES          # 256 rows per core
NB = R // 128            # 2 l-blocks
ND = D // 128            # 4 d-tiles

RUN_KWARGS = {}          # test harness can inject trace=True etc.
LAST_RESULTS = None
_PROGRAM_CACHE = {}


def _fp32r_round(x):
    u = np.ascontiguousarray(x, np.float32).view(np.uint32).astype(np.uint64)
    u = (u + 0x800) & 0xFFFFF000
    return (u & 0xFFFFFFFF).astype(np.uint32).view(np.float32)


def _build_program():
    nc = bacc.Bacc("TRN2", target_bir_lowering=False, debug=False,
                   num_devices=NCORES)

    # ---------------- DRAM I/O ----------------
    din = {}
    def inp(name, shape, dt=F32R):
        din[name] = nc.dram_tensor(name, list(shape), dt, kind="ExternalInput")
        return din[name]

    xT_d = inp("xT", [D, R])                 # x chunk transposed (rounded)
    x_d = inp("x_rm", [R, D], F32)           # residual, full fp32
    kw1_d = inp("ke_w1", [D, D])
    qw1_d = inp("qe_w1", [D, D])
    vw_d = inp("v_w", [D, D])
    ow_d = inp("w_eff", [D, D])
    w2k_d = inp("w2k", [D, 128])             # ke_w2 duplicated on cols
    w2q_d = inp("w2q", [D, 128])
    wam_d = inp("wamp", [D, 128])            # amp_w duplicated
    b1k_d = inp("b1k", [1, D])
    b1q_d = inp("b1q", [1, D])
    b2k_d = inp("b2k", [1, 128])
    b2q_d = inp("b2q", [1, 128])
    bam_d = inp("bamp", [1, 128])
    vb_d = inp("vb", [1, D])
    ob_d = inp("ob", [1, D])
    id_d = inp("ident", [128, 128])
    mask_d = inp("mask", [128, 128], F32)    # m[t,l] = 1 if l >= t
    wcol_d = inp("wcol", [128, 7], F32)      # per-core prefix 0/1 weights
    eps_d = inp("epsvec", [NB, 128], F32)    # 1e-5 * K * (gl+1), per l-block
    ones_d = inp("ones_r", [1, D])

    y_d = nc.dram_tensor("y", [R, D], F32, kind="ExternalOutput")

    with tile.TileContext(nc) as tc:
        with tc.tile_pool(name="sb", bufs=1) as sb, \
             tc.tile_pool(name="ps", bufs=1, space="PSUM") as ps, \
             tc.tile_pool(name="dr", bufs=1, space="DRAM") as dr:

            # ---------------- constants / small vectors ----------------
            ones_r = sb.tile([1, D], F32R, name="ones_r")
            nc.sync.dma_start(ones_r[:], ones_d[:])
            sinsc = sb.tile([128, 1], F32, name="sinsc")
            nc.gpsimd.memset(sinsc[0:64, :], -math.pi)
            nc.gpsimd.memset(sinsc[64:128, :], math.pi)
            sinbs = sb.tile([128, 1], F32, name="sinbs")
            nc.gpsimd.memset(sinbs[0:64, :], math.pi / 2)
            nc.gpsimd.memset(sinbs[64:128, :], 0.0)

            # ---------------- input loads ----------------
            def load_tiles(name, dram, p, f, n, dt=F32R):
                ts = []
                for t in range(n):
                    tl = sb.tile([p, f], dt, name=f"{name}{t}")
                    nc.sync.dma_start(tl[:], dram[t * p:(t + 1) * p, :])
                    ts.append(tl)
                return ts

            xT = load_tiles("xT", xT_d, 128, R, ND)
            kw1 = load_tiles("kw1", kw1_d, 128, D, ND)
            w2k = load_tiles("w2k", w2k_d, 128, 128, ND)
            wam = load_tiles("wam", wam_d, 128, 128, ND)
            vw = load_tiles("vw", vw_d, 128, D, ND)
            id_sb = sb.tile([128, 128], F32R, name="id_sb")
            nc.sync.dma_start(id_sb[:], id_d[:])
            b1k = sb.tile([1, D], F32R, name="b1k_sb")
            nc.sync.dma_start(b1k[:], b1k_d[:])
            b2k = sb.tile([1, 128], F32R, name="b2k_sb")
            nc.sync.dma_start(b2k[:], b2k_d[:])
            bam = sb.tile([1, 128], F32R, name="bam_sb")
            nc.sync.dma_start(bam[:], bam_d[:])
            vb = sb.tile([1, D], F32R, name="vb_sb")
            nc.sync.dma_start(vb[:], vb_d[:])
            qw1 = load_tiles("qw1", qw1_d, 128, D, ND)
            w2q = load_tiles("w2q", w2q_d, 128, 128, ND)
            b1q = sb.tile([1, D], F32R, name="b1q_sb")
            nc.sync.dma_start(b1q[:], b1q_d[:])
            b2q = sb.tile([1, 128], F32R, name="b2q_sb")
            nc.sync.dma_start(b2q[:], b2q_d[:])
            mask = sb.tile([128, 128], F32, name="mask_sb")
            nc.sync.dma_start(mask[:], mask_d[:])
            wcol = sb.tile([128, 7], F32, name="wcol_sb")
            nc.sync.dma_start(wcol[:], wcol_d[:])
            ow = load_tiles("ow", ow_d, 128, D, ND)
            ob = sb.tile([1, D], F32R, name="ob_sb")
            nc.sync.dma_start(ob[:], ob_d[:])
            x_rm = load_tiles("x_rm", x_d, 128, D, NB, dt=F32)
            epsv = []
            for lb in range(NB):
                ev = sb.tile([128, 1], F32, name=f"epsv{lb}")
                nc.sync.dma_start(ev[:], eps_d[lb:lb+1, :].rearrange("a b -> b a"))
                epsv.append(ev)

            # ---------------- k path (unblocks S + AllGather ASAP) ----------------
            hk = []
            for do in range(ND):
                hk_ps = ps.tile([128, R], F32, name=f"hk_ps{do}", tag="sm", bufs=2)
                for dj in range(ND):
                    nc.tensor.matmul(hk_ps[:], kw1[dj][:, do*128:(do+1)*128],
                                     xT[dj][:], start=(dj == 0), stop=False)
                nc.tensor.matmul(hk_ps[:], b1k[:, do*128:(do+1)*128],
                                 ones_r[:, 0:R], start=False, stop=True,
                                 skip_group_check=True)
                h_sb = sb.tile([128, R], F32R, name=f"hk{do}")
                nc.scalar.activation(h_sb[:], hk_ps[:], AF.Gelu)
                hk.append(h_sb)

            phk_ps = ps.tile([128, R], F32, name="phk_ps", tag="sm", bufs=2)
            for dj in range(ND):
                nc.tensor.matmul(phk_ps[:], w2k[dj][:], hk[dj][:],
                                 start=(dj == 0), stop=False)
            nc.tensor.matmul(phk_ps[:], b2k[:], ones_r[:, 0:R],
                             start=False, stop=True, skip_group_check=True)
            tk = sb.tile([128, R], F32, name="tk_sb")
            nc.scalar.activation(tk[:], phk_ps[:], AF.Tanh)
            nc.scalar.activation(tk[0:64, :], tk[0:64, :], AF.Abs)
            csk = sb.tile([128, R], F32, name="csk_sb")
            nc.scalar.activation(csk[:], tk[:], AF.Sin, bias=sinbs[:], scale=sinsc[:])

            # ---------------- amp (needed by phi_k and phi_q) ----------------
            am_ps = ps.tile([128, R], F32, name="am_ps", tag="sm", bufs=2)
            for dj in range(ND):
                nc.tensor.matmul(am_ps[:], wam[dj][:], xT[dj][:],
                                 start=(dj == 0), stop=False)
            nc.tensor.matmul(am_ps[:], bam[:], ones_r[:, 0:R],
                             start=False, stop=True, skip_group_check=True)
            e_sb = sb.tile([128, R], F32, name="e_sb")
            nc.scalar.activation(e_sb[:], am_ps[:], AF.Exp)
            e1_sb = sb.tile([128, R], F32, name="e1_sb")
            nc.vector.tensor_scalar_add(e1_sb[:], e_sb[:], 1.0)
            al_sb = sb.tile([128, R], F32, name="al_sb")
            nc.scalar.activation(al_sb[:], e1_sb[:], AF.Ln)

            phik = sb.tile([128, R], F32R, name="phik")
            nc.vector.scalar_tensor_tensor(phik[:], al_sb[:], 0.1, csk[:],
                                           ALU.add, ALU.mult)

            # ---------------- V ----------------
            V_sb = []
            for lb in range(NB):
                v_ps = ps.tile([128, D], F32, name=f"v_ps{lb}", tag="acc", bufs=2)
                for dj in range(ND):
                    nc.tensor.matmul(v_ps[:], xT[dj][:, lb*128:(lb+1)*128],
                                     vw[dj][:], start=(dj == 0), stop=False)
                nc.tensor.matmul(v_ps[:], ones_r[:, 0:128], vb[:], start=False,
                                 stop=True, skip_group_check=True)
                v_sb = sb.tile([128, D], F32R, name=f"V{lb}")
                nc.scalar.copy(v_sb[:], v_ps[:])
                V_sb.append(v_sb)

            # ---------------- chunk state S + AllGather (launch early) -----
            phik_rm = []
            for tb in range(NB):
                tr_ps = ps.tile([128, 128], F32, name=f"ktr_ps{tb}", tag="tr", bufs=2)
                nc.tensor.matmul(tr_ps[:], phik[:, tb*128:(tb+1)*128], id_sb[:],
                                 start=True, stop=True)
                k_rm = sb.tile([128, 128], F32R, name=f"phik_rm{tb}")
                nc.vector.tensor_copy(k_rm[:], tr_ps[:])
                phik_rm.append(k_rm)
            s_ps = ps.tile([128, D], F32, name="s_ps", tag="acc", bufs=2)
            for tb in range(NB):
                nc.tensor.matmul(s_ps[:], phik_rm[tb][:], V_sb[tb][:],
                                 start=(tb == 0), stop=(tb == NB - 1))
            s_sb = sb.tile([128, D], F32R, name="s_sb")
            nc.scalar.copy(s_sb[:], s_ps[:])
            cc_in = dr.tile([128, D], F32R, name="cc_in")
            cc_out = dr.tile([NCORES, 128, D], F32R, addr_space="Shared",
                             name="cc_out")
            nc.sync.dma_start(cc_in[:], s_sb[:])
            nc.gpsimd.collective_compute(
                "AllGather", ALU.bypass,
                replica_groups=[list(range(NCORES))],
                ins=[cc_in[:]], outs=[cc_out[:]],
            )

            # ---------------- q path (fills the AllGather window) ----------
            hq = []
            for do in range(ND):
                hq_ps = ps.tile([128, R], F32, name=f"hq_ps{do}", tag="sm", bufs=2)
                for dj in range(ND):
                    nc.tensor.matmul(hq_ps[:], qw1[dj][:, do*128:(do+1)*128],
                                     xT[dj][:], start=(dj == 0), stop=False)
                nc.tensor.matmul(hq_ps[:], b1q[:, do*128:(do+1)*128],
                                 ones_r[:, 0:R], start=False, stop=True,
                                 skip_group_check=True)
                h_sb = sb.tile([128, R], F32R, name=f"hq{do}")
                nc.scalar.activation(h_sb[:], hq_ps[:], AF.Gelu)
                hq.append(h_sb)
            phq_ps = ps.tile([128, R], F32, name="phq_ps", tag="sm", bufs=2)
            for dj in range(ND):
                nc.tensor.matmul(phq_ps[:], w2q[dj][:], hq[dj][:],
                                 start=(dj == 0), stop=False)
            nc.tensor.matmul(phq_ps[:], b2q[:], ones_r[:, 0:R],
                             start=False, stop=True, skip_group_check=True)
            tq = sb.tile([128, R], F32, name="tq_sb")
            nc.scalar.activation(tq[:], phq_ps[:], AF.Tanh)
            nc.scalar.activation(tq[0:64, :], tq[0:64, :], AF.Abs)
            csq = sb.tile([128, R], F32, name="csq_sb")
            nc.scalar.activation(csq[:], tq[:], AF.Sin, bias=sinbs[:], scale=sinsc[:])
            phiq = sb.tile([128, R], F32R, name="phiq")
            nc.vector.scalar_tensor_tensor(phiq[:], al_sb[:], 0.1, csq[:],
                                           ALU.add, ALU.mult)

            # ---------------- intra-chunk scores (overlap AG) ----------------
            a_m = {}
            for tb in range(NB):
                a_ps = ps.tile([128, R], F32, name=f"a_ps{tb}", tag="sm", bufs=2)
                nc.tensor.matmul(a_ps[:], phik[:, tb*128:(tb+1)*128], phiq[:],
                                 start=True, stop=True)
                if tb == 0:
                    a00 = sb.tile([128, 128], F32R, name="a00")
                    nc.vector.tensor_tensor(a00[:], a_ps[:, 0:128], mask[:], ALU.mult)
                    a01 = sb.tile([128, 128], F32R, name="a01")
                    nc.vector.tensor_copy(a01[:], a_ps[:, 128:256])
                    a_m[(0, 0)], a_m[(0, 1)] = a00, a01
                else:
                    a11 = sb.tile([128, 128], F32R, name="a11")
                    nc.vector.tensor_tensor(a11[:], a_ps[:, 128:256], mask[:], ALU.mult)
                    a_m[(1, 1)] = a11

            # ---------------- prefix state P (DVE; PE stays on attention) --
            s_all = []
            for j in range(NCORES - 1):
                sa = sb.tile([128, D], F32R, name=f"s_all{j}")
                nc.sync.dma_start(sa[:], cc_out[j])
                s_all.append(sa)
            p_acc = sb.tile([128, D], F32, name="p_acc")
            nc.vector.tensor_scalar_mul(p_acc[:], s_all[0][:], wcol[:, 0:1])
            for j in range(1, NCORES - 2):
                nc.vector.scalar_tensor_tensor(p_acc[:], s_all[j][:],
                                               wcol[:, j:j+1], p_acc[:],
                                               ALU.mult, ALU.add)
            p_sb = sb.tile([128, D], F32R, name="p_sb")
            nc.vector.scalar_tensor_tensor(p_sb[:], s_all[NCORES-2][:],
                                           wcol[:, NCORES-2:NCORES-1], p_acc[:],
                                           ALU.mult, ALU.add)
            # ---------------- retrieve + LN + out per l-block ----------------
            for lb in range(NB):
                r_ps = ps.tile([128, D], F32, name=f"r_ps{lb}", tag="racc", bufs=2)
                first = True
                for tb in range(lb + 1):
                    nc.tensor.matmul(r_ps[:], a_m[(tb, lb)][:], V_sb[tb][:],
                                     start=first, stop=False)
                    first = False
                nc.tensor.matmul(r_ps[:], phiq[:, lb*128:(lb+1)*128], p_sb[:],
                                 start=False, stop=True, skip_group_check=True)
                # LayerNorm stats (eps absorbs the 1/sqrt((l+1)K) row norm)
                bn6 = sb.tile([128, 6], F32, name=f"bn6_{lb}")
                nc.vector.bn_stats(bn6[:], r_ps[:])
                bn2 = sb.tile([128, 2], F32, name=f"bn2_{lb}")
                nc.vector.bn_aggr(bn2[:], bn6[:])
                lnv = sb.tile([128, 1], F32, name=f"lnv{lb}")
                nc.scalar.activation(lnv[:], bn2[:, 1:2], AF.Ln,
                                     bias=epsv[lb][:], scale=1.0)
                rstd = sb.tile([128, 1], F32, name=f"rstd{lb}")
                nc.scalar.activation(rstd[:], lnv[:], AF.Exp, bias=0.0, scale=-0.5)
                nmu = sb.tile([128, 1], F32, name=f"nmu{lb}")
                nc.vector.tensor_scalar_mul(nmu[:], bn2[:, 0:1], -1.0)
                s2v = sb.tile([128, 1], F32, name=f"s2v{lb}")
                nc.vector.tensor_tensor(s2v[:], nmu[:], rstd[:], ALU.mult)
                z_sb = sb.tile([128, D], F32R, name=f"z{lb}")
                nc.vector.tensor_scalar(z_sb[:], r_ps[:], rstd[:], s2v[:],
                                        ALU.mult, ALU.add)
                # transpose z, out-proj, bias, residual
                o_ps = ps.tile([128, D], F32, name=f"o_ps{lb}", tag="racc", bufs=2)
                for dt in range(ND):
                    zt_ps = ps.tile([128, 128], F32, name=f"zt_ps{lb}_{dt}",
                                    tag="tr", bufs=2)
                    nc.tensor.matmul(zt_ps[:], z_sb[:, dt*128:(dt+1)*128],
                                     id_sb[:], start=True, stop=True)
                    zt_sb = sb.tile([128, 128], F32R, name=f"zt{lb}_{dt}")
                    if dt % 2 == 0:
                        nc.vector.tensor_copy(zt_sb[:], zt_ps[:])
                    else:
                        nc.scalar.copy(zt_sb[:], zt_ps[:])
                    nc.tensor.matmul(o_ps[:], zt_sb[:], ow[dt][:],
                                     start=(dt == 0), stop=False,
                                     skip_group_check=True)
                nc.tensor.matmul(o_ps[:], ones_r[:, 0:128], ob[:], start=False,
                                 stop=True, skip_group_check=True)
                y_sb = sb.tile([128, D], F32, name=f"y{lb}")
                nc.vector.tensor_tensor(y_sb[:], o_ps[:], x_rm[lb][:], ALU.add)
                nc.sync.dma_start(y_d[lb*128:(lb+1)*128, :], y_sb[:])

    nc.compile()
    return nc


def kernel(**inputs):
    global LAST_RESULTS
    if 'prog' not in _PROGRAM_CACHE:
        _PROGRAM_CACHE['prog'] = _build_program()
    nc = _PROGRAM_CACHE['prog']

    f = {k: np.asarray(v, np.float32) for k, v in inputs.items()}
    x = f['x'][0]                                   # (L, D)
    rr = _fp32r_round
    W_eff = rr(f['ln_g'][:, None] * f['out_w'])
    b_eff = rr((f['ln_b'] @ f['out_w'] + f['out_b'])[None, :])
    shared = {
        "ke_w1": rr(f['ke_w1']), "qe_w1": rr(f['qe_w1']),
        "v_w": rr(f['v_w']), "w_eff": W_eff,
        "w2k": rr(np.concatenate([f['ke_w2'], f['ke_w2']], 1)),
        "w2q": rr(np.concatenate([f['qe_w2'], f['qe_w2']], 1)),
        "wamp": rr(np.concatenate([f['amp_w'], f['amp_w']], 1)),
        "b1k": rr(f['ke_b1'][None, :]), "b1q": rr(f['qe_b1'][None, :]),
        "b2k": rr(np.concatenate([f['ke_b2'], f['ke_b2']])[None, :]),
        "b2q": rr(np.concatenate([f['qe_b2'], f['qe_b2']])[None, :]),
        "bamp": rr(np.concatenate([f['amp_b'], f['amp_b']])[None, :]),
        "vb": rr(f['v_b'][None, :]), "ob": b_eff,
        "ident": np.eye(128, dtype=np.float32),
        "ones_r": np.ones((1, D), np.float32),
        "mask": (np.arange(128)[None, :] >= np.arange(128)[:, None]
                 ).astype(np.float32),
    }
    in_maps = []
    for c in range(NCORES):
        xc = x[R*c:R*(c+1)]
        wcol = np.zeros((128, 7), np.float32)
        wcol[:, :min(c, 7)] = 1.0
        gl = np.arange(R*c, R*(c+1), dtype=np.float64)
        in_maps.append({
            **shared,
            "xT": rr(np.ascontiguousarray(xc.T)),
            "x_rm": np.ascontiguousarray(xc),
            "wcol": wcol,
            "epsvec": (1e-5 * K * (gl + 1)).astype(np.float32).reshape(NB, 128),
        })

    res = run_bass_kernel_spmd(nc, in_maps, core_ids=list(range(NCORES)),
                               **RUN_KWARGS)
    LAST_RESULTS = res
    y = np.concatenate([res.results[c]['y'] for c in range(NCORES)], axis=0)
    return y[None].astype(np.float32)
